# revision 6
# baseline (speedup 1.0000x reference)
"""MoE head (N=65536, D=512, E=8, top-2) on 8 TRN2 NeuronCores — dense.

Data-parallel over tokens (8192/core). No gather/scatter: with E=8, K=2
dense all-expert compute is only 4x the dispatched FLOPs and keeps every
engine on fast, regular access patterns.

Per core, per 128-token tile:
  A. DMA x fp32 -> split bf16 hi/lo (DVE); PE-transpose both planes
     (bf16 is_transpose); 3-term split gate matmul (xh*Wh+xh*Wl+xl*Wh)
     -> exact-enough scores fp32 in PSUM; DVE max8/max_index top-2.
  A5. softmax over top-2 + dense gate weights gw[t,e] (DVE, whole-T).
  B. diag(gw_e) built by DVE (identity mask x per-partition scalar);
     scaled transpose x^T @ diag(gw_e) on PE (gating folded into the
     transpose); 33-matmul PSUM accumulation: gw^T @ bias + sum_e
     (gw_e*x)^T W_e^T -> token-major [128,512] fp32; ACT drain; DMA out.
"""

import numpy as np
import ml_dtypes
from contextlib import ExitStack

import concourse.bacc as bacc
import concourse.mybir as mybir
import concourse.tile as tile
from concourse.bass_utils import run_bass_kernel_spmd
from concourse.masks import make_identity

N, D, E, K = 65536, 512, 8, 2
NCORES = 8
T = N // NCORES            # 8192 tokens per core
NT = T // 128              # 64 token tiles
NQ = D // 128              # 4 k-quadrants

f32 = mybir.dt.float32
bf16 = mybir.dt.bfloat16
u32 = mybir.dt.uint32

_cached = {}


def build_nc():
    nc = bacc.Bacc("TRN2", target_bir_lowering=False)
    x_in = nc.dram_tensor("x", [T, D], f32, kind="ExternalInput")
    wt_in = nc.dram_tensor("wt", [128, E * NQ * D], bf16, kind="ExternalInput")
    wg_in = nc.dram_tensor("wg", [128, NQ * 2 * 8], bf16, kind="ExternalInput")
    bias_in = nc.dram_tensor("biasb", [8, D], bf16, kind="ExternalInput")
    bg_in = nc.dram_tensor("bg", [128, 8], f32, kind="ExternalInput")
    iota_in = nc.dram_tensor("iota8", [128, 8], f32, kind="ExternalInput")
    out = nc.dram_tensor("out", [T, D], f32, kind="ExternalOutput")

    x_r = x_in.rearrange("(j p) o -> p j o", p=128)     # [128, NT, 512]
    out_r = out.rearrange("(j p) o -> p j o", p=128)

    Copy = mybir.ActivationFunctionType.Copy

    with tile.TileContext(nc) as tc, ExitStack() as ctx:
        # -------- persistent buffers --------
        res = ctx.enter_context(tc.tile_pool(name="res", bufs=1))
        xh = res.tile([128, NT, D], bf16)          # 64 KiB/part
        wt_sb = res.tile([128, E * NQ * D], bf16)  # 32 KiB/part
        scoresT = res.tile([128, NT, 8], f32)
        maxv = res.tile([128, NT, 8], f32)
        argtop = res.tile([128, NT, 8], u32)
        gw = res.tile([128, NT, 8], f32)
        gwb = res.tile([128, NT, 8], bf16)
        w1c = res.tile([128, NT], f32)
        w2c = res.tile([128, NT], f32)
        wgt = res.tile([128, NQ * 2 * 8], bf16)
        bias_sb = res.tile([8, D], bf16)
        bg_sb = res.tile([128, 8], f32)
        iota_sb = res.tile([128, 8], f32)
        ident128 = res.tile([128, 128], f32)
        identb = res.tile([128, 128], bf16)
        make_identity(nc, ident128[:])
        make_identity(nc, identb[:])

        # weights/consts go on the scalar ring so the sync ring starts
        # streaming x tiles immediately; small tensors first, the 4 MiB
        # wt_sb last (not needed until phase B).
        nc.scalar.dma_start(wgt[:], wg_in[:])
        nc.scalar.dma_start(bg_sb[:], bg_in[:])
        nc.scalar.dma_start(iota_sb[:], iota_in[:])
        nc.scalar.dma_start(bias_sb[:], bias_in[:])
        nc.scalar.dma_start(wt_sb[:], wt_in[:])

        # phases A (gates) and B (experts) share one pool scope so the
        # second half of A can interleave with the first half of B: A's
        # DMA/DVE/ACT work hides under B's long PE stretches. PSUM is
        # squeezed to exactly 8 banks: psT(2) + psg(1) + psq(2) + pso(2)
        # + pgt(1).
        with tc.tile_pool(name="pa", bufs=4) as pa, \
             tc.tile_pool(name="px", bufs=8) as px, \
             tc.tile_pool(name="pa5", bufs=2) as pg5, \
             tc.tile_pool(name="pb", bufs=3) as pb, \
             tc.tile_pool(name="pxg", bufs=2) as pxg, \
             tc.tile_pool(name="ppt", bufs=2, space="PSUM") as ppt, \
             tc.tile_pool(name="ppg", bufs=1, space="PSUM") as ppg, \
             tc.tile_pool(name="ppq", bufs=2, space="PSUM") as ppq, \
             tc.tile_pool(name="ppo", bufs=2, space="PSUM") as ppo, \
             tc.tile_pool(name="ppw", bufs=1, space="PSUM") as ppw:
            # software-pipelined: PE transposes of tile j+1 are emitted
            # before the gate matmuls of tile j, so the PE queue never
            # stalls behind the ACT psum->sbuf copy of the current tile.
            xTs = {}

            def a_front(j):
                xt = px.tile([128, D], f32, tag="xt")
                nc.sync.dma_start(xt[:], x_r[:, j])
                nc.vector.tensor_copy(out=xh[:, j], in_=xt[:])
                xlt = pa.tile([128, D], bf16, tag="xlt")
                nc.gpsimd.tensor_sub(out=xlt[:], in0=xt[:], in1=xh[:, j])
                psT = ppt.tile([128, 8, 128], bf16, tag="psT")
                for q in range(NQ):
                    nc.tensor.transpose(
                        psT[:, q], xh[:, j, q * 128:(q + 1) * 128], identb[:])
                for q in range(NQ):
                    nc.tensor.transpose(
                        psT[:, 4 + q], xlt[:, q * 128:(q + 1) * 128], identb[:])
                xT = pa.tile([128, 8, 128], bf16, tag="xT")
                nc.scalar.activation(xT[:], psT[:], Copy)
                xTs[j] = xT

            def a_back(j):
                xT = xTs.pop(j)
                # gate: group A = xh*(Wh|Wl) 16-wide, group B = xl*Wh 8-wide
                psg = ppg.tile([128, 24], f32, tag="psg")
                for q in range(NQ):
                    nc.tensor.matmul(psg[:, 0:16], xT[:, q],
                                     wgt[:, q * 16:q * 16 + 16],
                                     start=(q == 0), stop=(q == NQ - 1))
                for q in range(NQ):
                    nc.tensor.matmul(psg[:, 16:24], xT[:, 4 + q],
                                     wgt[:, q * 16:q * 16 + 8],
                                     start=(q == 0), stop=(q == NQ - 1))
                nc.vector.tensor_add(out=scoresT[:, j], in0=psg[:, 0:8],
                                     in1=bg_sb[:])
                nc.vector.tensor_add(out=scoresT[:, j], in0=scoresT[:, j],
                                     in1=psg[:, 8:16])
                nc.vector.tensor_add(out=scoresT[:, j], in0=scoresT[:, j],
                                     in1=psg[:, 16:24])
                nc.vector.max(out=maxv[:, j], in_=scoresT[:, j])
                nc.vector.max_index(out=argtop[:, j], in_max=maxv[:, j],
                                    in_values=scoresT[:, j])

            def a5(h):
                # top-2 softmax + dense gate weights for one half of tiles
                s = slice(h * (NT // 2), (h + 1) * (NT // 2))
                HH = NT // 2
                dcol = pg5.tile([128, HH], f32, tag="dcol")
                ecol = pg5.tile([128, HH], f32, tag="ecol")
                nc.vector.tensor_sub(out=dcol[:], in0=maxv[:, s, 1],
                                     in1=maxv[:, s, 0])
                nc.scalar.activation(ecol[:], dcol[:],
                                     mybir.ActivationFunctionType.Exp)
                nc.vector.tensor_scalar_add(dcol[:], ecol[:], 1.0)
                nc.vector.reciprocal(w1c[:, s], dcol[:])
                nc.vector.tensor_mul(out=w2c[:, s], in0=ecol[:], in1=w1c[:, s])
                i1f = pg5.tile([128, HH], f32, tag="i1f")
                i2f = pg5.tile([128, HH], f32, tag="i2f")
                cmp1 = pg5.tile([128, HH, 8], f32, tag="cmp1")
                cmp2 = pg5.tile([128, HH, 8], f32, tag="cmp2")
                nc.vector.tensor_copy(out=i1f[:], in_=argtop[:, s, 0])
                nc.vector.tensor_copy(out=i2f[:], in_=argtop[:, s, 1])
                nc.vector.tensor_tensor(
                    out=cmp1[:],
                    in0=iota_sb[:, None, :].to_broadcast([128, HH, 8]),
                    in1=i1f[:, :, None].to_broadcast([128, HH, 8]),
                    op=mybir.AluOpType.is_equal)
                nc.vector.tensor_tensor(
                    out=cmp2[:],
                    in0=iota_sb[:, None, :].to_broadcast([128, HH, 8]),
                    in1=i2f[:, :, None].to_broadcast([128, HH, 8]),
                    op=mybir.AluOpType.is_equal)
                nc.vector.tensor_tensor(
                    out=cmp1[:], in0=cmp1[:],
                    in1=w1c[:, s, None].to_broadcast([128, HH, 8]),
                    op=mybir.AluOpType.mult)
                nc.vector.tensor_tensor(
                    out=cmp2[:], in0=cmp2[:],
                    in1=w2c[:, s, None].to_broadcast([128, HH, 8]),
                    op=mybir.AluOpType.mult)
                nc.vector.tensor_add(out=gw[:, s], in0=cmp1[:], in1=cmp2[:])
                nc.vector.tensor_copy(out=gwb[:, s], in_=gw[:, s])

            # software-pipelined: scaled transposes of tile j+1 are emitted
            # before the expert matmuls of tile j, so the PE queue keeps
            # working while the psum->sbuf copies of tile j drain.
            state = {}

            def b_front(j):
                dg = pb.tile([128, 8, 128], bf16, tag="dg")
                nc.vector.tensor_tensor(
                    out=dg[:],
                    in0=identb[:, None, :].to_broadcast([128, 8, 128]),
                    in1=gwb[:, j, :, None].to_broadcast([128, 8, 128]),
                    op=mybir.AluOpType.mult)
                pgt = ppw.tile([8, 128], bf16, tag="pgt")
                nc.tensor.transpose(pgt[:], gwb[:, j], identb[:])
                gwtb = pb.tile([8, 128], bf16, tag="gwtb")
                nc.scalar.activation(gwtb[:], pgt[:], Copy)
                xtg = pxg.tile([128, NQ, 8, 128], bf16, tag="xtg")
                for q in range(NQ):
                    xq = xh[:, j, q * 128:(q + 1) * 128]
                    for h in range(2):
                        psq = ppq.tile([128, 4, 128], f32, tag="psq")
                        nc.tensor.matmul(psq[:], xq, dg[:, 4 * h:4 * h + 4],
                                         start=True, stop=True)
                        dst = xtg[:, q, 4 * h:4 * h + 4]
                        if (2 * q + h) % 2 == 0:
                            nc.scalar.activation(dst, psq[:], Copy)
                        else:
                            nc.vector.tensor_copy(out=dst, in_=psq[:])
                state[j] = (gwtb, xtg)

            def b_back(j):
                gwtb, xtg = state.pop(j)
                pso = ppo.tile([128, D], f32, tag="pso")
                nc.tensor.matmul(pso[:], gwtb[:], bias_sb[:],
                                 start=True, stop=False)
                mm = 0
                for q in range(NQ):
                    for e in range(E):
                        nc.tensor.matmul(
                            pso[:], xtg[:, q, e],
                            wt_sb[:, (e * NQ + q) * D:(e * NQ + q + 1) * D],
                            start=False, stop=(mm == E * NQ - 1))
                        mm += 1
                outt = pb.tile([128, D], f32, tag="outt")
                nc.scalar.activation(outt[:], pso[:], Copy)
                if j % 2 == 0:
                    nc.sync.dma_start(out_r[:, j], outt[:])
                else:
                    nc.gpsimd.dma_start(out_r[:, j], outt[:])

            # --- emission schedule: A(first half) | A5(0) |
            #     A(second half) interleaved with B(first half) | A5(1) |
            #     B(second half) ---
            H = NT // 2
            for j in range(H):
                a_front(j)
                if j > 0:
                    a_back(j - 1)
            a_back(H - 1)
            a5(0)
            for i in range(H):
                j = H + i
                a_front(j)
                if i > 0:
                    a_back(j - 1)
                b_front(i)
                if i > 0:
                    b_back(i - 1)
            a_back(NT - 1)
            a5(1)
            b_front(H)
            b_back(H - 1)
            for i in range(H + 1, NT):
                b_front(i)
                b_back(i - 1)
            b_back(NT - 1)

    nc.compile()
    return nc


def _host_prep(W, b, Wg, bg):
    bf = ml_dtypes.bfloat16
    WT = np.ascontiguousarray(W.transpose(0, 2, 1)).astype(bf)  # [E, Din, Dout]
    wt = np.ascontiguousarray(
        WT.reshape(E, NQ, 128, D).transpose(2, 0, 1, 3)).reshape(128, E * NQ * D)
    WgT = np.ascontiguousarray(Wg.T.astype(np.float32))         # [512, 8]
    Wh = WgT.astype(bf)
    Wl = (WgT - Wh.astype(np.float32)).astype(bf)
    wg = np.zeros((128, NQ, 2, 8), dtype=bf)
    wg[:, :, 0, :] = Wh.reshape(NQ, 128, 8).transpose(1, 0, 2)
    wg[:, :, 1, :] = Wl.reshape(NQ, 128, 8).transpose(1, 0, 2)
    wg = np.ascontiguousarray(wg).reshape(128, NQ * 2 * 8)
    biasb = np.ascontiguousarray(b.astype(bf))
    bgv = np.tile(bg.astype(np.float32).reshape(1, 8), (128, 1))
    iota8 = np.tile(np.arange(8, dtype=np.float32), (128, 1))
    return wt, wg, biasb, bgv, iota8


def make_in_maps(inp):
    x = np.asarray(inp["x"], np.float32)
    wt, wg, biasb, bgv, iota8 = _host_prep(
        np.asarray(inp["W"], np.float32), np.asarray(inp["b"], np.float32),
        np.asarray(inp["Wg"], np.float32), np.asarray(inp["bg"], np.float32))
    in_maps = []
    for c in range(NCORES):
        in_maps.append({
            "x": np.ascontiguousarray(x[c * T:(c + 1) * T]),
            "wt": wt, "wg": wg, "biasb": biasb, "bg": bgv, "iota8": iota8,
        })
    return in_maps


def kernel(x, W, b, Wg, bg):
    if "nc" not in _cached:
        _cached["nc"] = build_nc()
    nc = _cached["nc"]
    in_maps = make_in_maps(dict(x=x, W=W, b=b, Wg=Wg, bg=bg))
    res = run_bass_kernel_spmd(nc, in_maps, core_ids=list(range(NCORES)))
    return np.concatenate([r["out"] for r in res.results], axis=0)


# revision 8
# speedup vs baseline: 1.0133x; 1.0133x over previous
"""MoE head (N=65536, D=512, E=8, top-2) on 8 TRN2 NeuronCores — dense.

Data-parallel over tokens (8192/core). No gather/scatter: with E=8, K=2
dense all-expert compute is only 4x the dispatched FLOPs and keeps every
engine on fast, regular access patterns.

Per core, per 128-token tile:
  A. DMA x fp32 -> split bf16 hi/lo (DVE); PE-transpose both planes
     (bf16 is_transpose); 3-term split gate matmul (xh*Wh+xh*Wl+xl*Wh)
     -> exact-enough scores fp32 in PSUM; DVE max8/max_index top-2.
  A5. softmax over top-2 + dense gate weights gw[t,e] (DVE, whole-T).
  B. diag(gw_e) built by DVE (identity mask x per-partition scalar);
     scaled transpose x^T @ diag(gw_e) on PE (gating folded into the
     transpose); 33-matmul PSUM accumulation: gw^T @ bias + sum_e
     (gw_e*x)^T W_e^T -> token-major [128,512] fp32; ACT drain; DMA out.
"""

import numpy as np
import ml_dtypes
from contextlib import ExitStack

import concourse.bacc as bacc
import concourse.mybir as mybir
import concourse.tile as tile
from concourse.bass_utils import run_bass_kernel_spmd
from concourse.masks import make_identity

N, D, E, K = 65536, 512, 8, 2
NCORES = 8
T = N // NCORES            # 8192 tokens per core
NT = T // 128              # 64 token tiles
NQ = D // 128              # 4 k-quadrants

f32 = mybir.dt.float32
bf16 = mybir.dt.bfloat16
u32 = mybir.dt.uint32

_cached = {}


def build_nc():
    nc = bacc.Bacc("TRN2", target_bir_lowering=False)
    x_in = nc.dram_tensor("x", [T, D], f32, kind="ExternalInput")
    wt_in = nc.dram_tensor("wt", [128, E * NQ * D], bf16, kind="ExternalInput")
    wg_in = nc.dram_tensor("wg", [128, NQ * 2 * 8], bf16, kind="ExternalInput")
    bias_in = nc.dram_tensor("biasb", [8, D], bf16, kind="ExternalInput")
    bg_in = nc.dram_tensor("bg", [128, 8], f32, kind="ExternalInput")
    iota_in = nc.dram_tensor("iota8", [128, 8], f32, kind="ExternalInput")
    out = nc.dram_tensor("out", [T, D], f32, kind="ExternalOutput")

    x_r = x_in.rearrange("(j p) o -> p j o", p=128)     # [128, NT, 512]
    out_r = out.rearrange("(j p) o -> p j o", p=128)

    Copy = mybir.ActivationFunctionType.Copy

    with tile.TileContext(nc) as tc, ExitStack() as ctx:
        # -------- persistent buffers --------
        res = ctx.enter_context(tc.tile_pool(name="res", bufs=1))
        xh = res.tile([128, NT, D], bf16)          # 64 KiB/part
        wt_sb = res.tile([128, E * NQ * D], bf16)  # 32 KiB/part
        scoresT = res.tile([128, NT, 8], f32)
        maxv = res.tile([128, NT, 8], f32)
        argtop = res.tile([128, NT, 8], u32)
        gw = res.tile([128, NT, 8], f32)
        gwb = res.tile([128, NT, 8], bf16)
        w1c = res.tile([128, NT], f32)
        w2c = res.tile([128, NT], f32)
        wgt = res.tile([128, NQ * 2 * 8], bf16)
        bias_sb = res.tile([8, D], bf16)
        bg_sb = res.tile([128, 8], f32)
        iota_sb = res.tile([128, 8], f32)
        ident128 = res.tile([128, 128], f32)
        identb = res.tile([128, 128], bf16)
        make_identity(nc, ident128[:])
        make_identity(nc, identb[:])

        # weights/consts go on the scalar ring so the sync ring starts
        # streaming x tiles immediately. The 4 MiB wt_sb is NOT loaded
        # here: its inline descriptor generation blocks whichever engine
        # queue issues it for ~13 us, so it is emitted in 4 chunks spread
        # across scalar/gpsimd between the first token tiles (it
        # is not needed until phase B).
        nc.scalar.dma_start(wgt[:], wg_in[:])
        nc.scalar.dma_start(bg_sb[:], bg_in[:])
        nc.scalar.dma_start(iota_sb[:], iota_in[:])
        nc.scalar.dma_start(bias_sb[:], bias_in[:])

        # phases A (gates) and B (experts) share one pool scope so the
        # second half of A can interleave with the first half of B: A's
        # DMA/DVE/ACT work hides under B's long PE stretches. PSUM is
        # squeezed to exactly 8 banks: psT(2) + psg(1) + psq(2) + pso(2)
        # + pgt(1).
        with tc.tile_pool(name="pa", bufs=4) as pa, \
             tc.tile_pool(name="px", bufs=8) as px, \
             tc.tile_pool(name="pa5", bufs=2) as pg5, \
             tc.tile_pool(name="pb", bufs=3) as pb, \
             tc.tile_pool(name="pxg", bufs=2) as pxg, \
             tc.tile_pool(name="ppt", bufs=2, space="PSUM") as ppt, \
             tc.tile_pool(name="ppg", bufs=1, space="PSUM") as ppg, \
             tc.tile_pool(name="ppq", bufs=2, space="PSUM") as ppq, \
             tc.tile_pool(name="ppo", bufs=2, space="PSUM") as ppo, \
             tc.tile_pool(name="ppw", bufs=1, space="PSUM") as ppw:
            # software-pipelined: PE transposes of tile j+1 are emitted
            # before the gate matmuls of tile j, so the PE queue never
            # stalls behind the ACT psum->sbuf copy of the current tile.
            xTs = {}

            def a_front(j):
                xt = px.tile([128, D], f32, tag="xt")
                nc.sync.dma_start(xt[:], x_r[:, j])
                nc.vector.tensor_copy(out=xh[:, j], in_=xt[:])
                xlt = pa.tile([128, D], bf16, tag="xlt")
                nc.gpsimd.tensor_sub(out=xlt[:], in0=xt[:], in1=xh[:, j])
                psT = ppt.tile([128, 8, 128], bf16, tag="psT")
                for q in range(NQ):
                    nc.tensor.transpose(
                        psT[:, q], xh[:, j, q * 128:(q + 1) * 128], identb[:])
                for q in range(NQ):
                    nc.tensor.transpose(
                        psT[:, 4 + q], xlt[:, q * 128:(q + 1) * 128], identb[:])
                xT = pa.tile([128, 8, 128], bf16, tag="xT")
                nc.scalar.activation(xT[:], psT[:], Copy)
                xTs[j] = xT

            def a_back(j):
                xT = xTs.pop(j)
                # gate: group A = xh*(Wh|Wl) 16-wide, group B = xl*Wh 8-wide
                psg = ppg.tile([128, 24], f32, tag="psg")
                for q in range(NQ):
                    nc.tensor.matmul(psg[:, 0:16], xT[:, q],
                                     wgt[:, q * 16:q * 16 + 16],
                                     start=(q == 0), stop=(q == NQ - 1))
                for q in range(NQ):
                    nc.tensor.matmul(psg[:, 16:24], xT[:, 4 + q],
                                     wgt[:, q * 16:q * 16 + 8],
                                     start=(q == 0), stop=(q == NQ - 1))
                nc.vector.tensor_add(out=scoresT[:, j], in0=psg[:, 0:8],
                                     in1=bg_sb[:])
                nc.vector.tensor_add(out=scoresT[:, j], in0=scoresT[:, j],
                                     in1=psg[:, 8:16])
                nc.vector.tensor_add(out=scoresT[:, j], in0=scoresT[:, j],
                                     in1=psg[:, 16:24])
                nc.vector.max(out=maxv[:, j], in_=scoresT[:, j])
                nc.vector.max_index(out=argtop[:, j], in_max=maxv[:, j],
                                    in_values=scoresT[:, j])

            def a5(h):
                # top-2 softmax + dense gate weights for one half of tiles
                s = slice(h * (NT // 2), (h + 1) * (NT // 2))
                HH = NT // 2
                dcol = pg5.tile([128, HH], f32, tag="dcol")
                ecol = pg5.tile([128, HH], f32, tag="ecol")
                nc.vector.tensor_sub(out=dcol[:], in0=maxv[:, s, 1],
                                     in1=maxv[:, s, 0])
                nc.scalar.activation(ecol[:], dcol[:],
                                     mybir.ActivationFunctionType.Exp)
                nc.vector.tensor_scalar_add(dcol[:], ecol[:], 1.0)
                nc.vector.reciprocal(w1c[:, s], dcol[:])
                nc.vector.tensor_mul(out=w2c[:, s], in0=ecol[:], in1=w1c[:, s])
                i1f = pg5.tile([128, HH], f32, tag="i1f")
                i2f = pg5.tile([128, HH], f32, tag="i2f")
                cmp1 = pg5.tile([128, HH, 8], f32, tag="cmp1")
                cmp2 = pg5.tile([128, HH, 8], f32, tag="cmp2")
                nc.vector.tensor_copy(out=i1f[:], in_=argtop[:, s, 0])
                nc.vector.tensor_copy(out=i2f[:], in_=argtop[:, s, 1])
                nc.vector.tensor_tensor(
                    out=cmp1[:],
                    in0=iota_sb[:, None, :].to_broadcast([128, HH, 8]),
                    in1=i1f[:, :, None].to_broadcast([128, HH, 8]),
                    op=mybir.AluOpType.is_equal)
                nc.vector.tensor_tensor(
                    out=cmp2[:],
                    in0=iota_sb[:, None, :].to_broadcast([128, HH, 8]),
                    in1=i2f[:, :, None].to_broadcast([128, HH, 8]),
                    op=mybir.AluOpType.is_equal)
                nc.vector.tensor_tensor(
                    out=cmp1[:], in0=cmp1[:],
                    in1=w1c[:, s, None].to_broadcast([128, HH, 8]),
                    op=mybir.AluOpType.mult)
                nc.vector.tensor_tensor(
                    out=cmp2[:], in0=cmp2[:],
                    in1=w2c[:, s, None].to_broadcast([128, HH, 8]),
                    op=mybir.AluOpType.mult)
                nc.vector.tensor_add(out=gw[:, s], in0=cmp1[:], in1=cmp2[:])
                nc.vector.tensor_copy(out=gwb[:, s], in_=gw[:, s])

            # software-pipelined: scaled transposes of tile j+1 are emitted
            # before the expert matmuls of tile j, so the PE queue keeps
            # working while the psum->sbuf copies of tile j drain.
            state = {}

            def b_front(j):
                dg = pb.tile([128, 8, 128], bf16, tag="dg")
                nc.vector.tensor_tensor(
                    out=dg[:],
                    in0=identb[:, None, :].to_broadcast([128, 8, 128]),
                    in1=gwb[:, j, :, None].to_broadcast([128, 8, 128]),
                    op=mybir.AluOpType.mult)
                pgt = ppw.tile([8, 128], bf16, tag="pgt")
                nc.tensor.transpose(pgt[:], gwb[:, j], identb[:])
                gwtb = pb.tile([8, 128], bf16, tag="gwtb")
                nc.scalar.activation(gwtb[:], pgt[:], Copy)
                xtg = pxg.tile([128, NQ, 8, 128], bf16, tag="xtg")
                for q in range(NQ):
                    xq = xh[:, j, q * 128:(q + 1) * 128]
                    for h in range(2):
                        psq = ppq.tile([128, 4, 128], f32, tag="psq")
                        nc.tensor.matmul(psq[:], xq, dg[:, 4 * h:4 * h + 4],
                                         start=True, stop=True)
                        dst = xtg[:, q, 4 * h:4 * h + 4]
                        if (2 * q + h) % 2 == 0:
                            nc.scalar.activation(dst, psq[:], Copy)
                        else:
                            nc.vector.tensor_copy(out=dst, in_=psq[:])
                state[j] = (gwtb, xtg)

            def b_back(j):
                gwtb, xtg = state.pop(j)
                pso = ppo.tile([128, D], f32, tag="pso")
                nc.tensor.matmul(pso[:], gwtb[:], bias_sb[:],
                                 start=True, stop=False)
                mm = 0
                for q in range(NQ):
                    for e in range(E):
                        nc.tensor.matmul(
                            pso[:], xtg[:, q, e],
                            wt_sb[:, (e * NQ + q) * D:(e * NQ + q + 1) * D],
                            start=False, stop=(mm == E * NQ - 1))
                        mm += 1
                outt = pb.tile([128, D], f32, tag="outt")
                nc.scalar.activation(outt[:], pso[:], Copy)
                if j % 2 == 0:
                    nc.sync.dma_start(out_r[:, j], outt[:])
                else:
                    nc.gpsimd.dma_start(out_r[:, j], outt[:])

            # --- emission schedule: A(first half) | A5(0) |
            #     A(second half) interleaved with B(first half) | A5(1) |
            #     B(second half) ---
            H = NT // 2
            WCH = E * NQ * D // 4
            wt_dma_engines = [nc.scalar, nc.gpsimd, nc.scalar, nc.gpsimd]
            for j in range(H):
                a_front(j)
                if j > 0:
                    a_back(j - 1)
                if j in (1, 3, 5, 7):
                    k = (j - 1) // 2
                    wt_dma_engines[k].dma_start(
                        wt_sb[:, k * WCH:(k + 1) * WCH],
                        wt_in[:, k * WCH:(k + 1) * WCH])
            a_back(H - 1)
            a5(0)
            for i in range(H):
                j = H + i
                a_front(j)
                if i > 0:
                    a_back(j - 1)
                b_front(i)
                if i > 0:
                    b_back(i - 1)
            a_back(NT - 1)
            a5(1)
            b_front(H)
            b_back(H - 1)
            for i in range(H + 1, NT):
                b_front(i)
                b_back(i - 1)
            b_back(NT - 1)

    nc.compile()
    return nc


def _host_prep(W, b, Wg, bg):
    bf = ml_dtypes.bfloat16
    WT = np.ascontiguousarray(W.transpose(0, 2, 1)).astype(bf)  # [E, Din, Dout]
    wt = np.ascontiguousarray(
        WT.reshape(E, NQ, 128, D).transpose(2, 0, 1, 3)).reshape(128, E * NQ * D)
    WgT = np.ascontiguousarray(Wg.T.astype(np.float32))         # [512, 8]
    Wh = WgT.astype(bf)
    Wl = (WgT - Wh.astype(np.float32)).astype(bf)
    wg = np.zeros((128, NQ, 2, 8), dtype=bf)
    wg[:, :, 0, :] = Wh.reshape(NQ, 128, 8).transpose(1, 0, 2)
    wg[:, :, 1, :] = Wl.reshape(NQ, 128, 8).transpose(1, 0, 2)
    wg = np.ascontiguousarray(wg).reshape(128, NQ * 2 * 8)
    biasb = np.ascontiguousarray(b.astype(bf))
    bgv = np.tile(bg.astype(np.float32).reshape(1, 8), (128, 1))
    iota8 = np.tile(np.arange(8, dtype=np.float32), (128, 1))
    return wt, wg, biasb, bgv, iota8


def make_in_maps(inp):
    x = np.asarray(inp["x"], np.float32)
    wt, wg, biasb, bgv, iota8 = _host_prep(
        np.asarray(inp["W"], np.float32), np.asarray(inp["b"], np.float32),
        np.asarray(inp["Wg"], np.float32), np.asarray(inp["bg"], np.float32))
    in_maps = []
    for c in range(NCORES):
        in_maps.append({
            "x": np.ascontiguousarray(x[c * T:(c + 1) * T]),
            "wt": wt, "wg": wg, "biasb": biasb, "bg": bgv, "iota8": iota8,
        })
    return in_maps


def kernel(x, W, b, Wg, bg):
    if "nc" not in _cached:
        _cached["nc"] = build_nc()
    nc = _cached["nc"]
    in_maps = make_in_maps(dict(x=x, W=W, b=b, Wg=Wg, bg=bg))
    res = run_bass_kernel_spmd(nc, in_maps, core_ids=list(range(NCORES)))
    return np.concatenate([r["out"] for r in res.results], axis=0)
